# revision 28
# baseline (speedup 1.0000x reference)
"""GCN context-paper kernel for 8 trn2 NeuronCores (SPMD via bass/Tile).

Model (see reference): proj+LN -> 3x GCNConv(+self loops, sym-norm) with
GELU -> concat(4 hops) -> MLP(GELU) -> LN.

Sharding: nodes partitioned across 8 cores (2500/core, padded to 2560).
Per hop: each core computes Y = h @ W for its nodes, AllGathers Y (fp8),
then builds its nodes' aggregation with indirect row-gathers of Y plus
one-hot matmuls on the tensor engine.

v3 notes (on top of v2):
- The GCN edge norm dis[s]*dis[d] is factorized: Y rows are pre-scaled
  by dis[src] (ACT per-partition scale at the agin write) and the
  aggregated rows post-scaled by dis[dst] (ACT scale at the psum->hn
  copy). The one-hot adjacency therefore holds exact binary values.
- Self loops are folded into the edge list (src=dst edges gathered from
  the AllGather output like any edge) - no special self chunks.
- The AllGather payload, gathered messages and one-hots are fp8 e4m3
  (exact for the binary one-hots; ~1e-2 final rel err from message
  quantization, vs the 2e-2 gate). Halves collective+gather HBM traffic.
- Scatter matmuls run in DoubleRow perf mode: one matmul contracts TWO
  128-row edge chunks (lhsT [128,2,128] onehots, rhs [128,2,N] gathered
  rows), halving PE instructions. Per-tile chunk counts are padded to
  even so pairs never straddle tiles/groups.
- Dense matmuls (proj/hop transform/MLP) stay fp16 (fp8 there fails the
  error budget; measured 4-7e-2).

DMA discipline: every DMA-queue instruction must end up with at most ONE
semaphore wait (hardware struct limit). Hence: DMA destinations in SBUF
are either fresh tiles or have engine-op (not DMA) prior writers; DMA
sources are external inputs or covered by dummy lane-warming DMAs
(collective output).
"""

import numpy as np
import ml_dtypes

import concourse.bass as bass
import concourse.bacc as bacc
import concourse.mybir as mybir
import concourse.tile as tile
from concourse.bass_utils import run_bass_kernel_spmd
from concourse.masks import make_identity

# problem constants (hardcoded per contract)
N, E, IN_F, H, HOPS = 20000, 100000, 1536, 768, 3
LN_EPS = 1e-5
NCORES = 8
NLOC = N // NCORES            # 2500 real nodes per core
P = 128
MT = 20                       # node tiles per core
NPAD = MT * P                 # 2560 padded nodes per core
HK = H // P                   # 6 feature tiles
INK = IN_F // P               # 12
CK = (HOPS + 1) * H // P      # 24 cat feature tiles
NSL = ((0, 512), (512, 256))  # N-dim slices for 768-wide outputs
OB = 8                        # chunks per one-hot load
GRP = 8                       # edge chunks per dma_gather

F32 = mybir.dt.float32
F16 = mybir.dt.float16
F8 = mybir.dt.float8e4
I32 = mybir.dt.int32
NF16 = np.float16
NF8 = ml_dtypes.float8_e4m3

# sim-only override: CoreSim requires one SWDGE queue per DMA semaphore;
# set to 0 to force all dma_gathers onto queue 0 when simulating.
GATHER_QUEUE = None
DR = mybir.MatmulPerfMode.DoubleRow
SPLIT_AG = True  # two AllGathers into strided column slices of one yg


# ---------------------------------------------------------------- host prep

def _prep(edge_index):
    """Host preprocessing: factorized normalization, self-loop folding,
    degree-balanced node->(core,tile,slot) assignment, per-core chunk
    tables (gather indices + binary one-hot blocks)."""
    src = np.asarray(edge_index[0], dtype=np.int64)
    dst = np.asarray(edge_index[1], dtype=np.int64)
    deg = np.bincount(dst, minlength=N).astype(np.float64) + 1.0
    dis = (1.0 / np.sqrt(deg)).astype(np.float64)

    # --- balance (indegree+1) load across the 160 (core,tile) bins so
    # every tile needs the same minimal chunk count
    load = np.bincount(dst, minlength=N).astype(np.int64) + 1
    NB = NCORES * MT
    order_n = np.argsort(-load, kind="stable")
    bin_load = np.zeros(NB, dtype=np.int64)
    bin_cnt = np.zeros(NB, dtype=np.int64)
    import heapq
    heap = [(0, 0, b) for b in range(NB)]  # (load, cnt, bin)
    heapq.heapify(heap)
    gnode = np.zeros(N, dtype=np.int64)  # node -> core*NPAD + tile*P + slot
    for n in order_n:
        ld, cnt, b = heapq.heappop(heap)
        core_b, tile_b = b // MT, b % MT
        gnode[n] = core_b * NPAD + tile_b * P + bin_cnt[b]
        bin_load[b] += load[n]
        bin_cnt[b] += 1
        if bin_cnt[b] < P:
            heapq.heappush(heap, (int(bin_load[b]), int(bin_cnt[b]), b))

    # fold self loops into the edge list
    alls = np.concatenate([src, np.arange(N, dtype=np.int64)])
    alld = np.concatenate([dst, np.arange(N, dtype=np.int64)])

    yg_row = gnode[alls]  # row in the AllGather output for each source
    gd = gnode[alld]
    core = gd // NPAD
    t = (gd % NPAD) // P
    d = gd % P  # slot within dst tile
    counts = np.zeros((NCORES, MT), dtype=np.int64)
    np.add.at(counts, (core, t), 1)
    # chunks per tile: max over cores (SPMD same program), padded to even
    c_list = []
    for tt in range(MT):
        c = max(1, int(np.ceil(counts[:, tt].max() / P)))
        c_list.append(c + (c & 1))
    off = np.zeros(MT, dtype=np.int64)
    off[1:] = np.cumsum(c_list)[:-1]
    nch = int(sum(c_list))

    oh = np.zeros((NCORES, nch * P, P), dtype=np.float32)

    order = np.lexsort((alls, t, core))  # stable ordering by (core, tile)
    so_core, so_t, so_d = core[order], t[order], d[order]
    so_yg = yg_row[order]
    grp = so_core * MT + so_t
    start = np.zeros(NCORES * MT + 1, dtype=np.int64)
    np.add.at(start, grp + 1, 1)
    start = np.cumsum(start)
    pos = np.arange(len(order)) - start[grp]
    chunk = off[so_t] + pos // P
    row = pos % P
    oh[so_core, chunk * P + row, so_d] = 1.0

    # int16 index stream for dma_gather
    ni_tot = nch * P
    idx16 = np.zeros((NCORES, 128, ni_tot // 16), dtype=np.int16)
    i_flat = chunk * P + row
    p16 = i_flat % 16
    c16 = i_flat // 16
    for cc in range(NCORES):
        m = so_core == cc
        a = np.zeros((16, ni_tot // 16), np.int16)
        a[p16[m], c16[m]] = so_yg[m].astype(np.int16)
        idx16[cc] = np.tile(a, (8, 1))

    # per-core dis column table [P, MT]: dis of the node assigned to
    # (core, tile, slot); 0 for empty slots so their Y rows zero out.
    discol = np.zeros((NCORES, P, MT), np.float32)
    gcore = gnode // NPAD
    gt_ = (gnode % NPAD) // P
    gs = gnode % P
    discol[gcore, gs, gt_] = dis
    return nch, c_list, oh.astype(NF8), idx16, discol, gnode


# --------------------------------------------------------------- bass build

def _build(nch, c_list, stage=4, fake_ag=False,
           triv_pb=False, triv_b2=False, triv_ln2=False):
    """Emit the SPMD Bass program. stage: 1=proj only, 2=+1 hop,
    3=+3 hops, 4=full (MLP+LN2). For stage<4 the output is the slab
    (feature-major) tiles of the last computed hop, [768, NPAD] f32.

    triv_pb/triv_b2/triv_ln2: skip ops that are numerically identity for
    the given inputs (proj_b==0, mlp_b2==0, ln2 affine==identity) -
    decided by the caller from the actual input values."""
    nc = bacc.Bacc(
        "TRN2", target_bir_lowering=False, debug=False, num_devices=NCORES,
        num_swdge_queues=4,
    )
    dp = nc.declare_dram_parameter
    xT = dp("xT", [P, MT * IN_F], F16, isOutput=False)
    projW = dp("projW", [IN_F, H], F16, isOutput=False)
    gcnW = dp("gcnW", [HOPS * H, H], F16, isOutput=False)
    # w1 packed by (block b, ktile, ftile): [P, 4*6*6*P]
    w1 = dp("w1", [P, (HOPS + 1) * HK * HK * P], F16, isOutput=False)
    w2 = dp("w2", [H, H], F16, isOutput=False)
    pbias = dp("pbias", [P, H], F32, isOutput=False)
    ln1gcol = dp("ln1gcol", [P, HK], F32, isOutput=False)
    ln1bcol = dp("ln1bcol", [P, HK], F32, isOutput=False)
    gbcol = dp("gbcol", [P, HOPS * HK], F32, isOutput=False)  # per-partition
    b1col = dp("b1col", [P, HK], F32, isOutput=False)
    b2 = dp("b2", [P, H], F32, isOutput=False)
    ln2g = dp("ln2g", [P, H], F32, isOutput=False)
    ln2b = dp("ln2b", [P, H], F32, isOutput=False)
    gidx = dp("gidx", [128, (nch * P) // 16], mybir.dt.int16, isOutput=False)
    ohw = dp("oh", [P, nch * P], F8, isOutput=False)
    discolp = dp("discol", [P, MT], F32, isOutput=False)

    nhop = 0 if stage <= 1 else (1 if stage == 2 else HOPS)
    if stage >= 4:
        out = dp("out", [NPAD, H], F32, isOutput=True)
    else:
        out = dp("out", [H, NPAD], F32, isOutput=True)

    off = np.zeros(MT, dtype=np.int64)
    off[1:] = np.cumsum(c_list)[:-1]

    with tile.TileContext(nc) as tc:
        import contextlib

        with contextlib.ExitStack() as ctx:
            dram = ctx.enter_context(tc.tile_pool(name="dram", bufs=1, space="DRAM"))
            cat = ctx.enter_context(tc.tile_pool(name="cat", bufs=1))
            cst = ctx.enter_context(tc.tile_pool(name="cst", bufs=1))

            # two rotating feature-major slabs (6 tiles each) + MLP1 acc
            slabs = [
                [cat.tile([P, NPAD], F16, name=f"s{s}_{f}") for f in range(HK)]
                for s in range(2)
            ]
            acc = [cat.tile([P, NPAD], F16, name=f"acc{f}") for f in range(HK)]

            idx_sb = cst.tile([128, (nch * P) // 16], mybir.dt.int16)
            nc.sync.dma_start(out=idx_sb[:], in_=gidx[:])
            gb_sb = cst.tile([P, HOPS * HK], F32)
            nc.sync.dma_start(out=gb_sb[:], in_=gbcol[:])
            l1g_sb = cst.tile([P, HK], F32)
            l1b_sb = cst.tile([P, HK], F32)
            nc.sync.dma_start(out=l1g_sb[:], in_=ln1gcol[:])
            nc.sync.dma_start(out=l1b_sb[:], in_=ln1bcol[:])
            dis_sb = cst.tile([P, MT], F32)
            nc.sync.dma_start(out=dis_sb[:], in_=discolp[:])
            ident = cst.tile([P, P], F16)
            make_identity(nc, ident[:])
            eps_t = cst.tile([P, 1], F32)
            nc.gpsimd.memset(eps_t[:], LN_EPS)

            def layer_norm(pool, src_ap, prefix):
                """Compute per-partition mean/rstd of [P, H] f32 src_ap.
                Returns (mu, rs) [P,1] tiles."""
                mu = pool.tile([P, 1], F32, tag=f"{prefix}mu", bufs=2)
                nc.vector.reduce_sum(out=mu[:], in_=src_ap, axis=mybir.AxisListType.X)
                nc.scalar.mul(out=mu[:], in_=mu[:], mul=1.0 / H)
                sqs = pool.tile([P, H], F16, tag=f"{prefix}sqs", bufs=2)
                ssum = pool.tile([P, 1], F32, tag=f"{prefix}ssum", bufs=2)
                nc.scalar.activation(
                    out=sqs[:], in_=src_ap,
                    func=mybir.ActivationFunctionType.Square,
                    accum_out=ssum[:],
                )
                mu2 = pool.tile([P, 1], F32, tag=f"{prefix}mu2", bufs=2)
                nc.vector.tensor_mul(out=mu2[:], in0=mu[:], in1=mu[:])
                # mu2 <- eps - mu^2
                nc.vector.tensor_scalar(
                    out=mu2[:], in0=mu2[:], scalar1=-1.0, scalar2=None,
                    op0=mybir.AluOpType.mult,
                )
                nc.vector.tensor_add(out=mu2[:], in0=mu2[:], in1=eps_t[:])
                rs = pool.tile([P, 1], F32, tag=f"{prefix}rs", bufs=2)
                nc.scalar.activation(
                    out=rs[:], in_=ssum[:],
                    func=mybir.ActivationFunctionType.Sqrt,
                    scale=1.0 / H, bias=mu2[:, :1],
                )
                nc.vector.reciprocal(out=rs[:], in_=rs[:])
                return mu, rs

            # ---------------- proj + LN1 -> slabs[0] (via PE transpose)
            with tc.tile_pool(name="proj", bufs=1) as pp, \
                    tc.tile_pool(name="psum_pj", bufs=1, space="PSUM") as psum:
                pw = [pp.tile([P, H], F16, name=f"pw{k}") for k in range(INK)]
                for k in range(INK):
                    nc.sync.dma_start(out=pw[k][:], in_=projW[k * P:(k + 1) * P, :])
                if not triv_pb:
                    pb_sb = pp.tile([P, H], F32)
                    nc.sync.dma_start(out=pb_sb[:], in_=pbias[:])

                for m in range(MT):
                    ms = slice(m * P, (m + 1) * P)
                    xs = pp.tile([P, INK, P], F16, tag="xslab", bufs=3)
                    nc.sync.dma_start(
                        out=xs[:],
                        in_=xT[:, m * IN_F:(m + 1) * IN_F].rearrange(
                            "p (k n) -> p k n", n=P
                        ),
                    )
                    ps = psum.tile([P, H], F32, tag="pj", bufs=3)
                    for n0, nn in NSL:
                        for k in range(INK):
                            nc.tensor.matmul(
                                out=ps[:, n0:n0 + nn],
                                lhsT=xs[:, k, :],
                                rhs=pw[k][:, n0:n0 + nn],
                                start=(k == 0),
                                stop=(k == INK - 1),
                            )
                    if triv_pb:
                        t0 = ps  # LN reads PSUM directly
                    else:
                        t0 = pp.tile([P, H], F32, tag="t0", bufs=2)
                        nc.vector.tensor_add(out=t0[:], in0=ps[:], in1=pb_sb[:])
                    mu, rs = layer_norm(pp, t0[:], "p")
                    h0 = pp.tile([P, H], F16, tag="h0", bufs=2)
                    nc.vector.tensor_scalar(
                        out=h0[:], in0=t0[:],
                        scalar1=mu[:, :1], scalar2=rs[:, :1],
                        op0=mybir.AluOpType.subtract, op1=mybir.AluOpType.mult,
                    )
                    # transpose 6 blocks -> slabs[0][f][:, m]; LN1 affine via ACT
                    for f in range(HK):
                        tp = psum.tile([P, P], F16, tag="tp", bufs=2)
                        nc.tensor.transpose(
                            out=tp[:], in_=h0[:, f * P:(f + 1) * P], identity=ident[:]
                        )
                        nc.scalar.activation(
                            out=slabs[0][f][:, ms], in_=tp[:],
                            func=mybir.ActivationFunctionType.Identity,
                            scale=l1g_sb[:, f:f + 1], bias=l1b_sb[:, f:f + 1],
                        )

            def load_w1b(b, pool):
                """Prefetch cat-block b's w1 slab (issue early: off the
                PE critical path at the hop boundary)."""
                w1b = pool.tile([P, HK * HK, P], F16, name=f"w1b{b}")
                nc.sync.dma_start(
                    out=w1b[:],
                    in_=w1[:, b * HK * HK * P:(b + 1) * HK * HK * P].rearrange(
                        "p (c n) -> p c n", n=P
                    ),
                )
                return w1b

            def mlp1_block(b, src, psum_m, w1b):
                """Accumulate cat-block b's MLP1 contribution into acc."""
                for n in range(5):
                    ns = slice(n * 512, (n + 1) * 512)
                    for f in range(HK):
                        pz = psum_m.tile([P, 512], F32, tag="z", bufs=2)
                        for kt in range(HK):
                            nc.tensor.matmul(
                                out=pz[:],
                                lhsT=w1b[:, kt * HK + f, :],
                                rhs=src[kt][:, ns],
                                start=(kt == 0),
                                stop=(kt == HK - 1),
                            )
                        if b == 0:
                            nc.vector.tensor_copy(out=acc[f][:, ns], in_=pz[:])
                        else:
                            nc.vector.tensor_add(
                                out=acc[f][:, ns], in0=acc[f][:, ns], in1=pz[:]
                            )

            def scatter_body(k, hpool, psum_s, yg, dst_s):
                oh_tiles = {}
                g_tiles = {}
                for t in range(MT):
                    ts = slice(t * P, (t + 1) * P)
                    pa = psum_s.tile([P, 512], F32, tag="sca", bufs=2)
                    pb_ = psum_s.tile([P, 256], F32, tag="scb", bufs=2)
                    for ci in range(0, c_list[t], 2):
                        ch = int(off[t]) + ci
                        gg, gj = ch // GRP, ch % GRP
                        if gj == 0:
                            ng = min(GRP, nch - gg * GRP)
                            gt = hpool.tile([P, ng, H], F8, tag="g",
                                            bufs=2, name=f"g{k}_{gg}")
                            nc.gpsimd.dma_gather(
                                out_ap=gt[:],
                                in_ap=yg[:],
                                idxs_ap=idx_sb[
                                    :, gg * GRP * 8:(gg * GRP + ng) * 8
                                ],
                                num_idxs=ng * P,
                                num_idxs_reg=ng * P,
                                elem_size=H,
                                queue_num=(gg % 4) if GATHER_QUEUE is None
                                else GATHER_QUEUE,
                            )
                            g_tiles[gg] = gt
                        og, oj = ch // OB, ch % OB
                        if oj == 0:
                            no = min(OB, nch - og * OB)
                            oh_t = hpool.tile([P, no, P], F8, tag="oh", bufs=3,
                                              name=f"oh{k}_{og}")
                            nc.sync.dma_start(
                                out=oh_t[:],
                                in_=ohw[
                                    :, og * OB * P:(og * OB + no) * P
                                ].rearrange("p (c m) -> p c m", m=P),
                            )
                            oh_tiles[og] = oh_t
                        oh_t = oh_tiles[og]
                        gt = g_tiles[gg]
                        first, last = (ci == 0), (ci + 2 >= c_list[t])
                        nc.tensor.matmul(
                            out=pa[:], lhsT=oh_t[:, oj:oj + 2, :],
                            rhs=gt[:, gj:gj + 2, 0:512],
                            start=first, stop=last, perf_mode=DR,
                        )
                        nc.tensor.matmul(
                            out=pb_[:], lhsT=oh_t[:, oj:oj + 2, :],
                            rhs=gt[:, gj:gj + 2, 512:H],
                            start=first, stop=last, perf_mode=DR,
                        )
                    # node-major psum -> dis[dst] scale (DVE; keeps ACT's
                    # func set parked on Gelu) -> fp16 -> PE transpose ->
                    # ACT (GCN bias + GELU) -> feature-major dst
                    hn = hpool.tile([P, H], F16, tag="hn", bufs=2)
                    nc.vector.tensor_scalar(
                        out=hn[:, 0:512], in0=pa[:],
                        scalar1=dis_sb[:, t:t + 1], scalar2=None,
                        op0=mybir.AluOpType.mult,
                    )
                    nc.vector.tensor_scalar(
                        out=hn[:, 512:H], in0=pb_[:],
                        scalar1=dis_sb[:, t:t + 1], scalar2=None,
                        op0=mybir.AluOpType.mult,
                    )
                    for f in range(HK):
                        tp = psum_s.tile([P, P], F16, tag="tp", bufs=4)
                        nc.tensor.transpose(
                            out=tp[:], in_=hn[:, f * P:(f + 1) * P],
                            identity=ident[:],
                        )
                        nc.scalar.activation(
                            out=dst_s[f][:, ts], in_=tp[:],
                            func=mybir.ActivationFunctionType.Gelu,
                            bias=gb_sb[:, k * HK + f:k * HK + f + 1],
                        )

            # ---------------- hops
            for k in range(nhop):
                src, dst_s = slabs[k % 2], slabs[(k + 1) % 2]
                hp = tc.tile_pool(name=f"hop{k}", bufs=1)
                with hp as hpool:
                    gw = [hpool.tile([P, H], F16, name=f"gw{k}_{f}") for f in range(HK)]
                    for f in range(HK):
                        nc.sync.dma_start(
                            out=gw[f][:], in_=gcnW[k * H + f * P:k * H + (f + 1) * P, :]
                        )
                    w1b = load_w1b(k, hpool) if stage >= 4 else None
                    if stage >= 4 and k == HOPS - 1:
                        # prefetch the tail's weights off the critical path
                        # (allocated from the long-lived pool)
                        tail_w1b = load_w1b(HOPS, cat)
                        tail_w2t = [cat.tile([P, H], F16, name=f"w2t{f}")
                                    for f in range(HK)]
                        for f in range(HK):
                            nc.sync.dma_start(
                                out=tail_w2t[f][:], in_=w2[f * P:(f + 1) * P, :]
                            )
                    # transform (NSL-block outer, node tile inner), then one
                    # AllGather of the full pre-scaled fp8 Y
                    yg = dram.tile(
                        [NCORES * NPAD, H], F8, addr_space="Shared",
                        name=f"yg{k}",
                    )
                    agin = dram.tile([NPAD, H], F8, name=f"agin{k}")
                    with tc.tile_pool(name=f"psum_t{k}", bufs=1, space="PSUM") as psum_t:
                        for bi, (n0, nn) in enumerate(NSL):
                            for m in range(MT):
                                ms = slice(m * P, (m + 1) * P)
                                ps = psum_t.tile([P, nn], F32, tag=f"y{bi}", bufs=2)
                                for f in range(HK):
                                    nc.tensor.matmul(
                                        out=ps[:],
                                        lhsT=src[f][:, ms],
                                        rhs=gw[f][:, n0:n0 + nn],
                                        start=(f == 0),
                                        stop=(f == HK - 1),
                                    )
                                # pre-scale by dis[node] and cast fp8
                                ya = hpool.tile([P, nn], F8, tag=f"ya{bi}", bufs=3)
                                nc.scalar.activation(
                                    out=ya[:], in_=ps[:],
                                    func=mybir.ActivationFunctionType.Copy,
                                    scale=dis_sb[:, m:m + 1],
                                )
                                nc.sync.dma_start(
                                    out=agin[ms, n0:n0 + nn], in_=ya[:]
                                )
                    if fake_ag:
                        # timing-proxy only: local copy standing in for the
                        # AllGather (the sim's collective model is ~12x
                        # pessimistic for intra-chip groups)
                        nc.sync.dma_start(out=yg[0:NPAD, :], in_=agin[:])
                    else:
                        nc.gpsimd.collective_compute(
                            "AllGather",
                            mybir.AluOpType.bypass,
                            ins=[agin.opt()],
                            outs=[yg.opt()],
                            replica_groups=[list(range(NCORES))],
                        )
                    # warm all 8 SWDGE lanes with 1-dep dummy reads of yg
                    for dlane in range(8):
                        dmy = hpool.tile([2, 4], F8, tag=f"dmy{dlane}", bufs=1)
                        nc.gpsimd.dma_start(
                            out=dmy[:], in_=yg[dlane * 2:dlane * 2 + 2, 0:4]
                        )
                    if k == 0:
                        # warm lanes on the idx region too (SBUF->SBUF tiny)
                        for dlane in range(8):
                            dmi = hpool.tile([2, 1], I32, tag=f"dmi{dlane}", bufs=1)
                            nc.gpsimd.dma_start(out=dmi[:], in_=idx_sb[dlane:dlane + 2, 0:1])

                    # MLP1 block for this hop's input slab - PE work that
                    # overlaps the AllGather
                    if stage >= 4:
                        with tc.tile_pool(name=f"psum_m{k}", bufs=1,
                                          space="PSUM") as psum_m:
                            mlp1_block(k, src, psum_m, w1b)

                    # flat chunk walk: grouped dma_gathers + batched onehots,
                    # node-major DoubleRow scatter
                    with contextlib.ExitStack() as sctx:
                        psum_s = sctx.enter_context(
                            tc.tile_pool(name=f"psum_s{k}", bufs=1, space="PSUM")
                        )
                        scatter_body(k, hpool, psum_s, yg, dst_s)

            if stage < 4:
                # dump last slab's tiles as [H, NPAD] f32
                last = slabs[nhop % 2]
                with tc.tile_pool(name="dump", bufs=1) as dpool:
                    for f in range(HK):
                        df = dpool.tile([P, NPAD], F32, tag="df", bufs=2)
                        nc.vector.tensor_copy(out=df[:], in_=last[f][:])
                        nc.sync.dma_start(out=out[f * P:(f + 1) * P, :], in_=df[:])

            if stage >= 4:
                # ---------------- MLP1 block 3 + MLP2 + LN2
                with tc.tile_pool(name="mlp", bufs=1) as mp, \
                        tc.tile_pool(name="psum_mlp", bufs=1, space="PSUM") as psum:
                    mlp1_block(HOPS, slabs[HOPS % 2], psum, tail_w1b)
                    w2t = tail_w2t
                    b1_sb = mp.tile([P, HK], F32)
                    nc.sync.dma_start(out=b1_sb[:], in_=b1col[:])
                    if not triv_b2:
                        b2_sb = mp.tile([P, H], F32)
                        nc.sync.dma_start(out=b2_sb[:], in_=b2[:])
                    if not triv_ln2:
                        l2g_sb = mp.tile([P, H], F32)
                        l2b_sb = mp.tile([P, H], F32)
                        nc.sync.dma_start(out=l2g_sb[:], in_=ln2g[:])
                        nc.sync.dma_start(out=l2b_sb[:], in_=ln2b[:])
                    # all GELUs first as 6 full-row ACT passes (one func-set
                    # load), then the matmul+LN walk runs Square/Sqrt only
                    zt = [mp.tile([P, NPAD], F16, name=f"zt{f}") for f in range(HK)]
                    for f in range(HK):
                        nc.scalar.activation(
                            out=zt[f][:], in_=acc[f][:],
                            func=mybir.ActivationFunctionType.Gelu,
                            bias=b1_sb[:, f:f + 1],
                        )
                    for m in range(MT):
                        mm = slice(m * P, (m + 1) * P)
                        po = psum.tile([P, H], F32, tag="o", bufs=3)
                        for n0, nn in NSL:
                            for f in range(HK):
                                nc.tensor.matmul(
                                    out=po[:, n0:n0 + nn],
                                    lhsT=zt[f][:, mm],
                                    rhs=w2t[f][:, n0:n0 + nn],
                                    start=(f == 0),
                                    stop=(f == HK - 1),
                                )
                        if triv_b2:
                            t0 = po  # LN reads PSUM directly
                        else:
                            t0 = mp.tile([P, H], F32, tag="t0", bufs=3)
                            nc.vector.tensor_add(out=t0[:], in0=po[:], in1=b2_sb[:])
                        mu, rs = layer_norm(mp, t0[:], "o")
                        t1 = mp.tile([P, H], F32, tag="t1", bufs=3)
                        nc.vector.tensor_scalar(
                            out=t1[:], in0=t0[:],
                            scalar1=mu[:, :1], scalar2=rs[:, :1],
                            op0=mybir.AluOpType.subtract,
                            op1=mybir.AluOpType.mult,
                        )
                        if not triv_ln2:
                            nc.vector.tensor_mul(out=t1[:], in0=t1[:], in1=l2g_sb[:])
                            nc.vector.tensor_add(out=t1[:], in0=t1[:], in1=l2b_sb[:])
                        nc.sync.dma_start(out=out[m * P:(m + 1) * P, :], in_=t1[:])
    nc.compile()
    return nc


def check_waits(nc, limit=1):
    """Return list of DMA-queue instructions exceeding the wait limit."""
    bad = []
    for f in nc.m.functions:
        for bb in f.blocks:
            for ins in bb.instructions:
                tn = type(ins).__name__
                if tn not in ("InstDMACopy", "InstDmaTransposeAnt"):
                    continue
                si = ins.sync_info
                if len(si.on_wait) > limit:
                    bad.append(
                        (ins.name, tn, str(ins.engine),
                         [(w.ant_name, w.wait_value) for w in si.on_wait])
                    )
    return bad


# ------------------------------------------------------------- entry point

def _in_maps(inputs, nch, oh, idx16, discol, gnode):
    x = np.asarray(inputs["x"], dtype=np.float32)
    xfull = np.zeros((NCORES * NPAD, IN_F), np.float32)
    xfull[gnode] = x
    bcast = lambda v: np.broadcast_to(
        np.asarray(v, np.float32), (P, H)
    ).copy()
    col = lambda v: np.asarray(v, np.float32).reshape(HK, P).T.copy()
    gb = np.asarray(inputs["gcn_b"], np.float32)  # [HOPS, H]
    gbcol = np.zeros((P, HOPS * HK), np.float32)
    for k in range(HOPS):
        gbcol[:, k * HK:(k + 1) * HK] = col(gb[k])
    w1 = np.asarray(inputs["mlp_w1"], np.float32)  # [3072, 768]
    # pack by (block b, ktile kt, ftile f): w1p[p, ((b*6+kt)*6+f)*128+n]
    w1p = (
        w1.reshape(HOPS + 1, HK, P, HK, P)   # [b, kt, p, f, n]
        .transpose(2, 0, 1, 3, 4)             # [p, b, kt, f, n]
        .reshape(P, (HOPS + 1) * HK * HK * P)
    )
    common = {
        "projW": np.asarray(inputs["proj_w"], np.float32).astype(NF16),
        "gcnW": np.asarray(inputs["gcn_w"], np.float32).reshape(HOPS * H, H).astype(NF16),
        "w1": w1p.astype(NF16),
        "w2": np.asarray(inputs["mlp_w2"], np.float32).astype(NF16),
        "pbias": bcast(inputs["proj_b"]),
        "ln1gcol": col(inputs["ln1_g"]),
        "ln1bcol": col(inputs["ln1_b"]),
        "gbcol": gbcol,
        "b1col": col(inputs["mlp_b1"]),
        "b2": bcast(inputs["mlp_b2"]),
        "ln2g": bcast(inputs["ln2_g"]),
        "ln2b": bcast(inputs["ln2_b"]),
    }
    maps = []
    for c in range(NCORES):
        xc = xfull[c * NPAD:(c + 1) * NPAD]
        # pack: xp[p, m*IN_F + k*128 + n] = x[m*128+n, k*128+p]
        xp = (
            xc.reshape(MT, P, INK, P)      # [m, n, k, p]
            .transpose(3, 0, 2, 1)          # [p, m, k, n]
            .reshape(P, MT * IN_F)
        )
        ohp = (
            oh[c].view(np.uint8).reshape(-1, P, P)  # [ch, p, d]
            .transpose(1, 0, 2)                      # [p, ch, d]
            .reshape(P, -1)
        ).view(NF8)
        m = dict(common)
        m["xT"] = xp.astype(NF16)
        m["gidx"] = idx16[c]
        m["oh"] = np.ascontiguousarray(ohp)
        m["discol"] = discol[c]
        maps.append(m)
    return maps


def _triv_flags(inputs):
    z = lambda v: not np.any(np.asarray(v))
    one = lambda v: bool(np.all(np.asarray(v) == 1.0))
    return dict(
        triv_pb=z(inputs["proj_b"]),
        triv_b2=z(inputs["mlp_b2"]),
        triv_ln2=one(inputs["ln2_g"]) and z(inputs["ln2_b"]),
    )


def kernel(**inputs):
    nch, c_list, oh, idx16, discol, gnode = _prep(np.asarray(inputs["edge_index"]))
    nc = _build(nch, c_list, stage=4, **_triv_flags(inputs))
    maps = _in_maps(inputs, nch, oh, idx16, discol, gnode)
    res = run_bass_kernel_spmd(nc, maps, list(range(NCORES)))
    allout = np.concatenate(
        [np.asarray(res.results[c]["out"]) for c in range(NCORES)], axis=0
    )
    return allout[gnode].astype(np.float32)
